# revision 3
# baseline (speedup 1.0000x reference)
"""Trainium2 Bass kernel for nn_ConcatRelationModule (gnn_message_passing).

Strategy: data-parallel over the edge dimension E across 8 NeuronCores.
The axon H2D link is the bottleneck, so the design minimizes bytes shipped
per call and the number of shipped tensors (each has fixed dispatch cost):
 - per-edge head rows fwd[gold_heads] are gathered on HOST (a pure
   permutation), modifier rows are the contiguous slice bwd[1:]; both ship
   as one fp16 feature-major tensor feat=[fwdT | bwdT] of [128, 2*E/8] per
   core (~134MB total instead of replicating the 1GB fwd table).
 - all MLP weights ship as one replicated fp16 blob, all biases + per-core
   gold relation labels as one f32 blob.
 - per 512-edge tile on device: 3-layer MLP on the PE (fp16 in, f32
   accumulate), tanh/bias on ScalarE, hinge (gold vs best-wrong label) on
   VectorE; lerrs transposed back edge-major on the PE, output as bf16.
"""
import sys

sys.path.insert(0, "/opt/trn_rl_repo")

import numpy as np

import concourse.bass as bass
import concourse.bacc as bacc
import concourse.mybir as mybir
import concourse.tile as tile
from concourse.bass_utils import run_bass_kernel_spmd
from concourse.masks import make_identity

F32 = mybir.dt.float32
F16 = mybir.dt.float16
BF16 = mybir.dt.bfloat16

N = 262144
L = 128
H = 128
H2 = 128
R = 64
E = N - 1
NCORES = 8
EPC = N // NCORES            # edges per core (very last edge is padding)
NB = EPC // 128              # 256 blocks of 128 edges
NT = EPC // 512              # 64 tiles of 512 edges
WCOLS = 832                  # 6x128 (wfoh/wfom/rh2 halves) + 64 (rout)
BCOLS = 4 + NB               # 4 bias cols + per-core rels


def build_kernel():
    nc = bacc.Bacc("TRN2", target_bir_lowering=False, debug=False)

    feat_d = nc.declare_dram_parameter("feat", [128, 2 * EPC], F16, isOutput=False)
    wall_d = nc.declare_dram_parameter("wall", [128, WCOLS], F16, isOutput=False)
    ball_d = nc.declare_dram_parameter("ball", [128, BCOLS], F32, isOutput=False)

    lerr_d = nc.declare_dram_parameter("lerr", [EPC], BF16, isOutput=True)
    lerr_v = lerr_d[:].rearrange("(b p) -> b p", p=128)

    with tile.TileContext(nc) as tc:
        with (
            tc.tile_pool(name="const", bufs=1) as cp,
            tc.tile_pool(name="work", bufs=3) as wp,
            tc.tile_pool(name="ps", bufs=1, space="PSUM") as pp,
            tc.tile_pool(name="ps2", bufs=2, space="PSUM") as pp2,
        ):
            # ---- constants ----
            ident = cp.tile([128, 128], F32, tag="ident")
            make_identity(nc, ident[:])

            wall = cp.tile([128, WCOLS], F16, tag="wall")
            nc.sync.dma_start(out=wall[:], in_=wall_d[:])
            wfoh_f = wall[:, 0:128]
            wfoh_b = wall[:, 128:256]
            wfom_f = wall[:, 256:384]
            wfom_b = wall[:, 384:512]
            rh2_a = wall[:, 512:640]
            rh2_b = wall[:, 640:768]
            rout_t = wall[:, 768:832]

            ball = cp.tile([128, BCOLS], F32, tag="ball")
            nc.sync.dma_start(out=ball[:], in_=ball_d[:])
            bias_h = ball[:, 0:1]
            bias_m = ball[:, 1:2]
            bias_2 = ball[:, 2:3]
            bias_r = ball[0:64, 3:4]
            rels_sb = ball[:, 4:4 + NB]

            iota_t = cp.tile([128, 4 * R], F32, tag="iota")
            nc.gpsimd.iota(
                out=iota_t[:].rearrange("p (j r) -> p j r", r=R),
                pattern=[[0, 4], [1, R]],
                channel_multiplier=0,
                allow_small_or_imprecise_dtypes=True,
            )

            lerr_acc = cp.tile([128, NB], F32, tag="lerr_acc")

            # ---- main pipeline ----
            for t in range(NT):
                fwdT_t = wp.tile([128, 512], F16, tag="fwdT_t")
                nc.sync.dma_start(out=fwdT_t[:], in_=feat_d[:, t * 512:(t + 1) * 512])
                bwdT_t = wp.tile([128, 512], F16, tag="bwdT_t")
                nc.sync.dma_start(
                    out=bwdT_t[:], in_=feat_d[:, EPC + t * 512:EPC + (t + 1) * 512])

                fov = pp.tile([128, 512], F32, tag="fov")
                nc.tensor.matmul(out=fov[:], lhsT=wfoh_f, rhs=fwdT_t[:],
                                 start=True, stop=False)
                nc.tensor.matmul(out=fov[:], lhsT=wfoh_b, rhs=bwdT_t[:],
                                 start=False, stop=True)
                h1 = wp.tile([128, 512], F16, tag="h1")
                nc.scalar.activation(
                    out=h1[:], in_=fov[:],
                    func=mybir.ActivationFunctionType.Tanh,
                    bias=bias_h,
                )

                mov = pp.tile([128, 512], F32, tag="mov")
                nc.tensor.matmul(out=mov[:], lhsT=wfom_f, rhs=fwdT_t[:],
                                 start=True, stop=False)
                nc.tensor.matmul(out=mov[:], lhsT=wfom_b, rhs=bwdT_t[:],
                                 start=False, stop=True)
                h1m = wp.tile([128, 512], F16, tag="h1m")
                nc.scalar.activation(
                    out=h1m[:], in_=mov[:],
                    func=mybir.ActivationFunctionType.Tanh,
                    bias=bias_m,
                )

                h2p = pp.tile([128, 512], F32, tag="h2p")
                nc.tensor.matmul(out=h2p[:], lhsT=rh2_a, rhs=h1[:],
                                 start=True, stop=False)
                nc.tensor.matmul(out=h2p[:], lhsT=rh2_b, rhs=h1m[:],
                                 start=False, stop=True)
                h2s = wp.tile([128, 512], F16, tag="h2s")
                nc.scalar.activation(
                    out=h2s[:], in_=h2p[:],
                    func=mybir.ActivationFunctionType.Tanh,
                    bias=bias_2,
                )

                scp = pp2.tile([64, 512], F32, tag="scp")
                nc.tensor.matmul(out=scp[:], lhsT=rout_t, rhs=h2s[:],
                                 start=True, stop=True)
                ssb = wp.tile([64, 512], F32, tag="ssb")
                nc.scalar.activation(
                    out=ssb[:], in_=scp[:],
                    func=mybir.ActivationFunctionType.Identity,
                    bias=bias_r,
                )

                # scores back to [edge, label] layout
                stp = pp.tile([128, 4 * R], F32, tag="stp")
                for k in range(4):
                    nc.tensor.transpose(
                        out=stp[:, k * R:(k + 1) * R],
                        in_=ssb[:, k * 128:(k + 1) * 128],
                        identity=ident[0:64, 0:64],
                    )
                st3 = stp[:].rearrange("p (j r) -> p j r", r=R)

                # hinge on VectorE
                relx = rels_sb[:, 4 * t:4 * t + 4].to_broadcast([128, 4, R])
                mask = wp.tile([128, 4 * R], F32, tag="mask")
                nc.vector.tensor_tensor(
                    out=mask[:].rearrange("p (j r) -> p j r", r=R),
                    in0=iota_t[:].rearrange("p (j r) -> p j r", r=R),
                    in1=relx,
                    op=mybir.AluOpType.is_equal,
                )
                m3 = mask[:].rearrange("p (j r) -> p j r", r=R)
                gmul = wp.tile([128, 4 * R], F32, tag="gmul")
                nc.vector.tensor_tensor(
                    out=gmul[:].rearrange("p (j r) -> p j r", r=R),
                    in0=st3, in1=m3, op=mybir.AluOpType.mult,
                )
                gold = wp.tile([128, 4], F32, tag="gold")
                nc.vector.reduce_sum(
                    out=gold[:], in_=gmul[:].rearrange("p (j r) -> p j r", r=R),
                    axis=mybir.AxisListType.X,
                )
                wm = wp.tile([128, 4 * R], F32, tag="wm")
                nc.vector.scalar_tensor_tensor(
                    out=wm[:].rearrange("p (j r) -> p j r", r=R),
                    in0=m3, scalar=-1e30, in1=st3,
                    op0=mybir.AluOpType.mult, op1=mybir.AluOpType.add,
                )
                wrong = wp.tile([128, 4], F32, tag="wrong")
                nc.vector.reduce_max(
                    out=wrong[:], in_=wm[:].rearrange("p (j r) -> p j r", r=R),
                    axis=mybir.AxisListType.X,
                )
                dtile = wp.tile([128, 4], F32, tag="dtile")
                nc.vector.tensor_tensor(
                    out=dtile[:], in0=wrong[:], in1=gold[:],
                    op=mybir.AluOpType.subtract,
                )
                nc.vector.scalar_tensor_tensor(
                    out=lerr_acc[:, 4 * t:4 * t + 4],
                    in0=dtile[:], scalar=-1.0, in1=dtile[:],
                    op0=mybir.AluOpType.is_gt, op1=mybir.AluOpType.mult,
                )

            # ---- write out lerrs (transpose to edge-major, bf16) ----
            for a in range(0, NB, 128):
                otp = pp.tile([128, 128], F32, tag="otp")
                nc.tensor.transpose(
                    out=otp[:],
                    in_=lerr_acc[:, a:a + 128],
                    identity=ident[:],
                )
                osb = wp.tile([128, 128], BF16, tag="osb")
                nc.scalar.copy(out=osb[:], in_=otp[:])
                nc.sync.dma_start(out=lerr_v[a:a + 128, :], in_=osb[:])

    nc.compile()
    return nc


_NC_CACHE = {}


def _get_nc():
    if "nc" not in _NC_CACHE:
        _NC_CACHE["nc"] = build_kernel()
    return _NC_CACHE["nc"]


def prepare_weights(WFOH, WFOM, rcatBias, rhid2Layer, rhid2Bias, routLayer,
                    routBias):
    """Pack all MLP weights into one fp16 [128, WCOLS] blob (wall) and the
    biases into the first 4 columns of the f32 ball blob."""
    wall = np.empty((128, WCOLS), dtype=np.float16)
    wall[:, 0:128] = np.asarray(WFOH, np.float16)[0:128]
    wall[:, 128:256] = np.asarray(WFOH, np.float16)[128:256]
    wall[:, 256:384] = np.asarray(WFOM, np.float16)[0:128]
    wall[:, 384:512] = np.asarray(WFOM, np.float16)[128:256]
    wall[:, 512:640] = np.asarray(rhid2Layer, np.float16)[0:128]
    wall[:, 640:768] = np.asarray(rhid2Layer, np.float16)[128:256]
    wall[:, 768:832] = np.asarray(routLayer, np.float16)

    bcols = np.zeros((128, 4), dtype=np.float32)
    bcat = np.asarray(rcatBias, np.float32).reshape(-1)
    bcols[:, 0] = bcat[0:128]
    bcols[:, 1] = bcat[128:256]
    bcols[:, 2] = np.asarray(rhid2Bias, np.float32).reshape(-1)
    bcols[0:64, 3] = np.asarray(routBias, np.float32).reshape(-1)
    return dict(wall=wall, bcols=bcols)


def prepare_core_inputs(fwd, bwd, gold_heads, gold_rels, weights):
    """Host-side prep: gather head rows, slice modifier rows, fp16-ify,
    transpose to feature-major per-core tiles."""
    fwd16 = np.asarray(fwd, dtype=np.float16)
    bwd16 = np.asarray(bwd, dtype=np.float16)

    heads_pad = np.zeros(N, dtype=np.int64)
    heads_pad[:E] = np.asarray(gold_heads, dtype=np.int64)
    rels_pad = np.zeros(N, dtype=np.int64)
    rels_pad[:E] = np.asarray(gold_rels, dtype=np.int64)

    fwd_g = fwd16[heads_pad]                        # [N, 128] gathered head rows
    bwd_g = np.empty((N, L), dtype=np.float16)      # [N, 128] modifier rows
    bwd_g[:E] = bwd16[1:]
    bwd_g[E] = 0

    in_maps = []
    for c in range(NCORES):
        sl = slice(c * EPC, (c + 1) * EPC)
        feat = np.empty((128, 2 * EPC), dtype=np.float16)
        feat[:, :EPC] = fwd_g[sl].T
        feat[:, EPC:] = bwd_g[sl].T
        ball = np.empty((128, BCOLS), dtype=np.float32)
        ball[:, 0:4] = weights["bcols"]
        ball[:, 4:] = rels_pad[sl].astype(np.float32).reshape(NB, 128).T
        in_maps.append(dict(feat=feat, wall=weights["wall"], ball=ball))
    return in_maps


def assemble_output(results):
    return np.concatenate(
        [np.asarray(results[c]["lerr"]).astype(np.float32) for c in range(NCORES)]
    )[:E]


def kernel(fwd, bwd, gold_heads, gold_rels, WFOH, WFOM, rhidBias, rcatBias,
           rhid2Layer, rhid2Bias, routLayer, routBias):
    nc = _get_nc()
    weights = prepare_weights(WFOH, WFOM, rcatBias, rhid2Layer, rhid2Bias,
                              routLayer, routBias)
    in_maps = prepare_core_inputs(fwd, bwd, gold_heads, gold_rels, weights)
    res = run_bass_kernel_spmd(nc, in_maps, list(range(NCORES)))
    return assemble_output(res.results)


# revision 8
# speedup vs baseline: 1.1713x; 1.1713x over previous
"""Trainium2 Bass kernel for nn_ConcatRelationModule (gnn_message_passing).

Strategy: shard edges across 8 NeuronCores by the HEAD's table shard
(core c gets edges whose gold_head lies in rows [c*32K, (c+1)*32K)), so each
core only needs head rows from its own shard — and only the ~20.8K UNIQUE
referenced rows are shipped (fp16), with per-edge int16 indices resolved by
an on-device dma_gather. Modifier rows (bwd[e+1]) are host-gathered into the
bucketed edge order and shipped fp16 feature-major. This minimizes bytes over
the slow axon H2D link: ~116MB/call vs 1.2GB for the replicated-table
baseline.

Device per 512-edge tile: PE-transpose of gathered head rows, 3-layer MLP on
the PE (fp16 in, f32 accumulate), tanh/bias on ScalarE, hinge (gold vs
best-wrong label) on VectorE; lerrs transposed edge-major, output bf16, and
scattered back to natural edge order on host.
"""
import sys

sys.path.insert(0, "/opt/trn_rl_repo")

import numpy as np

import concourse.bass as bass
import concourse.bacc as bacc
import concourse.mybir as mybir
import concourse.tile as tile
from concourse.bass_utils import run_bass_kernel_spmd
from concourse.masks import make_identity

F32 = mybir.dt.float32
F16 = mybir.dt.float16
BF16 = mybir.dt.bfloat16
I16 = mybir.dt.int16

N = 262144
L = 128
H = 128
H2 = 128
R = 64
E = N - 1
NCORES = 8
SHARD = N // NCORES          # head-table rows owned per core (32768)
CAP = 33280                  # per-core edge capacity (65 tiles of 512)
UCAP = 21504                 # per-core unique-head-row capacity (168*128)
NB = CAP // 128              # 260 blocks of 128 edges
NT = CAP // 512              # 65 tiles of 512 edges
GSPLITS = [2048] * 16 + [512]    # dma_gather split sizes, sum == CAP
WCOLS = 832                  # 6x128 (wfoh/wfom/rh2 halves) + 64 (rout)
BCOLS = 4 + NB               # 4 bias cols + per-core rels


def build_kernel():
    nc = bacc.Bacc("TRN2", target_bir_lowering=False, debug=False)

    fwdu_d = nc.declare_dram_parameter("fwdu", [UCAP, 128], F16, isOutput=False)
    bwdT_d = nc.declare_dram_parameter("bwdT", [128, CAP], F16, isOutput=False)
    idx_d = nc.declare_dram_parameter("idx", [128, CAP // 16], I16, isOutput=False)
    wall_d = nc.declare_dram_parameter("wall", [128, WCOLS], F16, isOutput=False)
    ball_d = nc.declare_dram_parameter("ball", [128, BCOLS], F32, isOutput=False)

    lerr_d = nc.declare_dram_parameter("lerr", [CAP], BF16, isOutput=True)
    lerr_v = lerr_d[:].rearrange("(b p) -> b p", p=128)

    with tile.TileContext(nc) as tc:
        with (
            tc.tile_pool(name="const", bufs=1) as cp,
            tc.tile_pool(name="gath", bufs=4) as gp,
            tc.tile_pool(name="work", bufs=3) as wp,
            tc.tile_pool(name="ps", bufs=1, space="PSUM") as pp,
            tc.tile_pool(name="ps2", bufs=2, space="PSUM") as pp2,
        ):
            # ---- constants ----
            ident = cp.tile([128, 128], F32, tag="ident")
            make_identity(nc, ident[:])
            ident16 = cp.tile([128, 128], F16, tag="ident16")
            nc.vector.tensor_copy(out=ident16[:], in_=ident[:])

            wall = cp.tile([128, WCOLS], F16, tag="wall")
            nc.sync.dma_start(out=wall[:], in_=wall_d[:])
            wfoh_f = wall[:, 0:128]
            wfoh_b = wall[:, 128:256]
            wfom_f = wall[:, 256:384]
            wfom_b = wall[:, 384:512]
            rh2_a = wall[:, 512:640]
            rh2_b = wall[:, 640:768]
            rout_t = wall[:, 768:832]

            ball = cp.tile([128, BCOLS], F32, tag="ball")
            nc.sync.dma_start(out=ball[:], in_=ball_d[:])
            bias_h = ball[:, 0:1]
            bias_m = ball[:, 1:2]
            bias_2 = ball[:, 2:3]
            bias_r = ball[0:64, 3:4]
            rels_sb = ball[:, 4:4 + NB]

            idx_sb = cp.tile([128, CAP // 16], I16, tag="idx_sb")
            nc.sync.dma_start(out=idx_sb[:], in_=idx_d[:])

            iota_t = cp.tile([128, 4 * R], F32, tag="iota")
            nc.gpsimd.iota(
                out=iota_t[:].rearrange("p (j r) -> p j r", r=R),
                pattern=[[0, 4], [1, R]],
                channel_multiplier=0,
                allow_small_or_imprecise_dtypes=True,
            )

            lerr_acc = cp.tile([128, NB], F32, tag="lerr_acc")

            # ---- main pipeline ----
            t_global = 0
            slot = 0
            for gn in GSPLITS:
                fwdg = gp.tile([128, 2048], F16, tag="fwdg")
                nc.gpsimd.dma_gather(
                    out_ap=fwdg[:, 0:gn].rearrange("p (j e) -> p j e", e=128),
                    in_ap=fwdu_d[:],
                    idxs_ap=idx_sb[:, slot // 16:(slot + gn) // 16],
                    num_idxs=gn,
                    num_idxs_reg=gn,
                    elem_size=128,
                    single_packet=False,
                )
                for ti in range(gn // 512):
                    t = t_global
                    off = ti * 512
                    # transpose gathered head rows -> [feature, edge]
                    tp = pp.tile([128, 512], F16, tag="tp")
                    for k in range(4):
                        nc.tensor.transpose(
                            out=tp[:, k * 128:(k + 1) * 128],
                            in_=fwdg[:, off + k * 128:off + (k + 1) * 128],
                            identity=ident16[:],
                        )
                    fwdT_t = wp.tile([128, 512], F16, tag="fwdT_t")
                    nc.scalar.copy(out=fwdT_t[:], in_=tp[:])

                    bwdT_t = wp.tile([128, 512], F16, tag="bwdT_t")
                    nc.sync.dma_start(
                        out=bwdT_t[:], in_=bwdT_d[:, t * 512:(t + 1) * 512])

                    fov = pp.tile([128, 512], F32, tag="fov")
                    nc.tensor.matmul(out=fov[:], lhsT=wfoh_f, rhs=fwdT_t[:],
                                     start=True, stop=False)
                    nc.tensor.matmul(out=fov[:], lhsT=wfoh_b, rhs=bwdT_t[:],
                                     start=False, stop=True)
                    h1 = wp.tile([128, 512], F16, tag="h1")
                    nc.scalar.activation(
                        out=h1[:], in_=fov[:],
                        func=mybir.ActivationFunctionType.Tanh,
                        bias=bias_h,
                    )

                    mov = pp.tile([128, 512], F32, tag="mov")
                    nc.tensor.matmul(out=mov[:], lhsT=wfom_f, rhs=fwdT_t[:],
                                     start=True, stop=False)
                    nc.tensor.matmul(out=mov[:], lhsT=wfom_b, rhs=bwdT_t[:],
                                     start=False, stop=True)
                    h1m = wp.tile([128, 512], F16, tag="h1m")
                    nc.scalar.activation(
                        out=h1m[:], in_=mov[:],
                        func=mybir.ActivationFunctionType.Tanh,
                        bias=bias_m,
                    )

                    h2p = pp.tile([128, 512], F32, tag="h2p")
                    nc.tensor.matmul(out=h2p[:], lhsT=rh2_a, rhs=h1[:],
                                     start=True, stop=False)
                    nc.tensor.matmul(out=h2p[:], lhsT=rh2_b, rhs=h1m[:],
                                     start=False, stop=True)
                    h2s = wp.tile([128, 512], F16, tag="h2s")
                    nc.scalar.activation(
                        out=h2s[:], in_=h2p[:],
                        func=mybir.ActivationFunctionType.Tanh,
                        bias=bias_2,
                    )

                    scp = pp2.tile([64, 512], F32, tag="scp")
                    nc.tensor.matmul(out=scp[:], lhsT=rout_t, rhs=h2s[:],
                                     start=True, stop=True)
                    ssb = wp.tile([64, 512], F32, tag="ssb")
                    nc.scalar.activation(
                        out=ssb[:], in_=scp[:],
                        func=mybir.ActivationFunctionType.Identity,
                        bias=bias_r,
                    )

                    # scores back to [edge, label] layout
                    stp = pp.tile([128, 4 * R], F32, tag="stp")
                    for k in range(4):
                        nc.tensor.transpose(
                            out=stp[:, k * R:(k + 1) * R],
                            in_=ssb[:, k * 128:(k + 1) * 128],
                            identity=ident[0:64, 0:64],
                        )
                    st3 = stp[:].rearrange("p (j r) -> p j r", r=R)

                    # hinge on VectorE
                    relx = rels_sb[:, 4 * t:4 * t + 4].to_broadcast([128, 4, R])
                    mask = wp.tile([128, 4 * R], F32, tag="mask")
                    nc.vector.tensor_tensor(
                        out=mask[:].rearrange("p (j r) -> p j r", r=R),
                        in0=iota_t[:].rearrange("p (j r) -> p j r", r=R),
                        in1=relx,
                        op=mybir.AluOpType.is_equal,
                    )
                    m3 = mask[:].rearrange("p (j r) -> p j r", r=R)
                    gmul = wp.tile([128, 4 * R], F32, tag="gmul")
                    nc.vector.tensor_tensor(
                        out=gmul[:].rearrange("p (j r) -> p j r", r=R),
                        in0=st3, in1=m3, op=mybir.AluOpType.mult,
                    )
                    gold = wp.tile([128, 4], F32, tag="gold")
                    nc.vector.reduce_sum(
                        out=gold[:], in_=gmul[:].rearrange("p (j r) -> p j r", r=R),
                        axis=mybir.AxisListType.X,
                    )
                    wm = wp.tile([128, 4 * R], F32, tag="wm")
                    nc.vector.scalar_tensor_tensor(
                        out=wm[:].rearrange("p (j r) -> p j r", r=R),
                        in0=m3, scalar=-1e30, in1=st3,
                        op0=mybir.AluOpType.mult, op1=mybir.AluOpType.add,
                    )
                    wrong = wp.tile([128, 4], F32, tag="wrong")
                    nc.vector.reduce_max(
                        out=wrong[:], in_=wm[:].rearrange("p (j r) -> p j r", r=R),
                        axis=mybir.AxisListType.X,
                    )
                    dtile = wp.tile([128, 4], F32, tag="dtile")
                    nc.vector.tensor_tensor(
                        out=dtile[:], in0=wrong[:], in1=gold[:],
                        op=mybir.AluOpType.subtract,
                    )
                    nc.vector.scalar_tensor_tensor(
                        out=lerr_acc[:, 4 * t:4 * t + 4],
                        in0=dtile[:], scalar=-1.0, in1=dtile[:],
                        op0=mybir.AluOpType.is_gt, op1=mybir.AluOpType.mult,
                    )
                    t_global += 1
                slot += gn

            # ---- write out lerrs (transpose to edge-major, bf16) ----
            for a in range(0, NB, 128):
                cols = min(128, NB - a)
                otp = pp.tile([128, 128], F32, tag="otp")
                nc.tensor.transpose(
                    out=otp[0:cols, :],
                    in_=lerr_acc[:, a:a + cols],
                    identity=ident[:],
                )
                osb = wp.tile([128, 128], BF16, tag="osb")
                nc.scalar.copy(out=osb[0:cols, :], in_=otp[0:cols, :])
                nc.sync.dma_start(out=lerr_v[a:a + cols, :], in_=osb[0:cols, :])

    nc.compile()
    return nc


_NC_CACHE = {}


def _get_nc():
    if "nc" not in _NC_CACHE:
        _NC_CACHE["nc"] = build_kernel()
    return _NC_CACHE["nc"]


def prepare_weights(WFOH, WFOM, rcatBias, rhid2Layer, rhid2Bias, routLayer,
                    routBias):
    """Pack all MLP weights into one fp16 [128, WCOLS] blob (wall) and the
    biases into the first 4 columns of the f32 ball blob."""
    wall = np.empty((128, WCOLS), dtype=np.float16)
    wall[:, 0:128] = np.asarray(WFOH, np.float16)[0:128]
    wall[:, 128:256] = np.asarray(WFOH, np.float16)[128:256]
    wall[:, 256:384] = np.asarray(WFOM, np.float16)[0:128]
    wall[:, 384:512] = np.asarray(WFOM, np.float16)[128:256]
    wall[:, 512:640] = np.asarray(rhid2Layer, np.float16)[0:128]
    wall[:, 640:768] = np.asarray(rhid2Layer, np.float16)[128:256]
    wall[:, 768:832] = np.asarray(routLayer, np.float16)

    bcols = np.zeros((128, 4), dtype=np.float32)
    bcat = np.asarray(rcatBias, np.float32).reshape(-1)
    bcols[:, 0] = bcat[0:128]
    bcols[:, 1] = bcat[128:256]
    bcols[:, 2] = np.asarray(rhid2Bias, np.float32).reshape(-1)
    bcols[0:64, 3] = np.asarray(routBias, np.float32).reshape(-1)
    return dict(wall=wall, bcols=bcols)


def prepare_core_inputs(fwd, bwd, gold_heads, gold_rels, weights):
    """Bucket edges by head shard; dedup head rows per core; gather modifier
    rows into bucket order. Returns (in_maps, edge_lists) where edge_lists[c]
    maps the core's slot i -> global edge id (for output scatter)."""
    fwd16 = np.asarray(fwd, dtype=np.float16)
    bwd16 = np.asarray(bwd, dtype=np.float16)
    heads = np.asarray(gold_heads, dtype=np.int64)
    rels = np.asarray(gold_rels, dtype=np.int64)
    shard = heads >> 15

    in_maps = []
    edge_lists = []
    for c in range(NCORES):
        edges_c = np.nonzero(shard == c)[0]
        cnt = len(edges_c)
        if cnt > CAP:
            raise OverflowError(f"core {c}: {cnt} edges > CAP {CAP}")

        u, inv = np.unique(heads[edges_c], return_inverse=True)
        if len(u) > UCAP:
            raise OverflowError(f"core {c}: {len(u)} unique rows > UCAP {UCAP}")
        fwdu = np.zeros((UCAP, 128), dtype=np.float16)
        fwdu[:len(u)] = fwd16[u]

        idx_local = np.zeros(CAP, dtype=np.int16)
        idx_local[:cnt] = inv.astype(np.int16)
        idx16 = np.tile(np.ascontiguousarray(idx_local.reshape(CAP // 16, 16).T),
                        (8, 1))

        bwdT = np.zeros((128, CAP), dtype=np.float16)
        bwdT[:, :cnt] = bwd16[edges_c + 1].T

        ball = np.empty((128, BCOLS), dtype=np.float32)
        ball[:, 0:4] = weights["bcols"]
        rels_pad = np.zeros(CAP, dtype=np.float32)
        rels_pad[:cnt] = rels[edges_c]
        ball[:, 4:] = rels_pad.reshape(NB, 128).T

        in_maps.append(dict(fwdu=fwdu, bwdT=bwdT, idx=idx16,
                            wall=weights["wall"], ball=ball))
        edge_lists.append(edges_c)
    return in_maps, edge_lists


def assemble_output(results, edge_lists):
    lerr_full = np.zeros(E, dtype=np.float32)
    for c in range(NCORES):
        out = np.asarray(results[c]["lerr"]).astype(np.float32)
        lerr_full[edge_lists[c]] = out[:len(edge_lists[c])]
    return lerr_full


def kernel(fwd, bwd, gold_heads, gold_rels, WFOH, WFOM, rhidBias, rcatBias,
           rhid2Layer, rhid2Bias, routLayer, routBias):
    nc = _get_nc()
    weights = prepare_weights(WFOH, WFOM, rcatBias, rhid2Layer, rhid2Bias,
                              routLayer, routBias)
    in_maps, edge_lists = prepare_core_inputs(fwd, bwd, gold_heads, gold_rels,
                                              weights)
    res = run_bass_kernel_spmd(nc, in_maps, list(range(NCORES)))
    return assemble_output(res.results, edge_lists)


# revision 9
# speedup vs baseline: 1.1870x; 1.0134x over previous
"""Trainium2 Bass kernel for nn_ConcatRelationModule (gnn_message_passing).

Strategy: shard edges across 8 NeuronCores by the HEAD's table shard
(core c gets edges whose gold_head lies in rows [c*32K, (c+1)*32K)), so each
core only needs head rows from its own shard — and only the ~20.8K UNIQUE
referenced rows are shipped (fp16), with per-edge int16 indices resolved by
an on-device dma_gather. Modifier rows (bwd[e+1]) are host-gathered into the
bucketed edge order and shipped fp16 feature-major.

The axon H2D link is the bottleneck (~45-55MB/s) and each input tensor adds
~12ms of fixed dispatch cost, so ALL per-core inputs are packed into ONE
fp16 blob (~14.1MB/core, ~113MB total vs 1.2GB for the replicated-table
baseline); int16 indices and f32 biases/labels ride in it as raw bit
patterns and are recovered on device via AP bitcast.

Device per 512-edge tile: dma_gather head rows, PE-transpose, 3-layer MLP on
the PE (fp16 in, f32 accumulate), tanh/bias on ScalarE, hinge (gold vs
best-wrong label) on VectorE; lerrs transposed edge-major, output bf16, and
scattered back to natural edge order on host.
"""
import sys

sys.path.insert(0, "/opt/trn_rl_repo")

import numpy as np

import concourse.bass as bass
import concourse.bacc as bacc
import concourse.mybir as mybir
import concourse.tile as tile
from concourse.bass_utils import run_bass_kernel_spmd
from concourse.masks import make_identity

F32 = mybir.dt.float32
F16 = mybir.dt.float16
BF16 = mybir.dt.bfloat16
I16 = mybir.dt.int16

N = 262144
L = 128
H = 128
H2 = 128
R = 64
E = N - 1
NCORES = 8
SHARD = N // NCORES          # head-table rows owned per core (32768)
CAP = 33280                  # per-core edge capacity (65 tiles of 512)
UCAP = 20864                 # per-core unique-head-row capacity (163*128)
NB = CAP // 128              # 260 blocks of 128 edges
NT = CAP // 512              # 65 tiles of 512 edges
GSPLITS = [2048] * 16 + [512]    # dma_gather split sizes, sum == CAP
WCOLS = 832                  # 6x128 (wfoh/wfom/rh2 halves) + 64 (rout)
BCOLS = 4 + NB               # 4 bias cols + per-core rels (f32)
IDXC = CAP // 16             # idx cols (int16, replicated to 128 partitions)

# blob layout, in fp16-element offsets
OFF_BWDT = 0
OFF_FWDU = OFF_BWDT + 128 * CAP
OFF_IDX = OFF_FWDU + UCAP * 128
OFF_WALL = OFF_IDX + 128 * IDXC
OFF_BALL = OFF_WALL + 128 * WCOLS
TOT = OFF_BALL + 128 * 2 * BCOLS


def build_kernel():
    nc = bacc.Bacc("TRN2", target_bir_lowering=False, debug=False)

    blob_d = nc.declare_dram_parameter("blob", [TOT], F16, isOutput=False)
    lerr_d = nc.declare_dram_parameter("lerr", [CAP], BF16, isOutput=True)
    lerr_v = lerr_d[:].rearrange("(b p) -> b p", p=128)

    bwdT2d = blob_d[OFF_BWDT:OFF_BWDT + 128 * CAP].rearrange("(p x) -> p x", x=CAP)
    fwdu2d = blob_d[OFF_FWDU:OFF_FWDU + UCAP * 128].rearrange("(r f) -> r f", f=128)
    idx2d = blob_d[OFF_IDX:OFF_IDX + 128 * IDXC].rearrange("(p x) -> p x", x=IDXC)
    wall2d = blob_d[OFF_WALL:OFF_WALL + 128 * WCOLS].rearrange("(p x) -> p x", x=WCOLS)
    ball2d = blob_d[OFF_BALL:].rearrange("(p x) -> p x", x=2 * BCOLS)

    with tile.TileContext(nc) as tc:
        with (
            tc.tile_pool(name="const", bufs=1) as cp,
            tc.tile_pool(name="gath", bufs=4) as gp,
            tc.tile_pool(name="work", bufs=3) as wp,
            tc.tile_pool(name="ps", bufs=1, space="PSUM") as pp,
            tc.tile_pool(name="ps2", bufs=2, space="PSUM") as pp2,
        ):
            # ---- constants ----
            ident = cp.tile([128, 128], F32, tag="ident")
            make_identity(nc, ident[:])
            ident16 = cp.tile([128, 128], F16, tag="ident16")
            nc.vector.tensor_copy(out=ident16[:], in_=ident[:])

            wall = cp.tile([128, WCOLS], F16, tag="wall")
            nc.sync.dma_start(out=wall[:], in_=wall2d)
            wfoh_f = wall[:, 0:128]
            wfoh_b = wall[:, 128:256]
            wfom_f = wall[:, 256:384]
            wfom_b = wall[:, 384:512]
            rh2_a = wall[:, 512:640]
            rh2_b = wall[:, 640:768]
            rout_t = wall[:, 768:832]

            ballraw = cp.tile([128, 2 * BCOLS], F16, tag="ballraw")
            nc.sync.dma_start(out=ballraw[:], in_=ball2d)
            ball = cp.tile([128, BCOLS], F32, tag="ball")
            nc.vector.tensor_copy(out=ball[:], in_=ballraw[:].bitcast(F32))
            bias_h = ball[:, 0:1]
            bias_m = ball[:, 1:2]
            bias_2 = ball[:, 2:3]
            bias_r = ball[0:64, 3:4]
            rels_sb = ball[:, 4:4 + NB]

            idx_sb = cp.tile([128, IDXC], F16, tag="idx_sb")
            nc.sync.dma_start(out=idx_sb[:], in_=idx2d)
            idx_i16 = idx_sb[:].bitcast(I16)

            iota_t = cp.tile([128, 4 * R], F32, tag="iota")
            nc.gpsimd.iota(
                out=iota_t[:].rearrange("p (j r) -> p j r", r=R),
                pattern=[[0, 4], [1, R]],
                channel_multiplier=0,
                allow_small_or_imprecise_dtypes=True,
            )

            lerr_acc = cp.tile([128, NB], F32, tag="lerr_acc")

            # ---- main pipeline ----
            t_global = 0
            slot = 0
            for gn in GSPLITS:
                fwdg = gp.tile([128, 2048], F16, tag="fwdg")
                nc.gpsimd.dma_gather(
                    out_ap=fwdg[:, 0:gn].rearrange("p (j e) -> p j e", e=128),
                    in_ap=fwdu2d,
                    idxs_ap=idx_i16[:, slot // 16:(slot + gn) // 16],
                    num_idxs=gn,
                    num_idxs_reg=gn,
                    elem_size=128,
                    single_packet=False,
                )
                for ti in range(gn // 512):
                    t = t_global
                    off = ti * 512
                    # transpose gathered head rows -> [feature, edge]
                    tp = pp.tile([128, 512], F16, tag="tp")
                    for k in range(4):
                        nc.tensor.transpose(
                            out=tp[:, k * 128:(k + 1) * 128],
                            in_=fwdg[:, off + k * 128:off + (k + 1) * 128],
                            identity=ident16[:],
                        )
                    fwdT_t = wp.tile([128, 512], F16, tag="fwdT_t")
                    nc.scalar.copy(out=fwdT_t[:], in_=tp[:])

                    bwdT_t = wp.tile([128, 512], F16, tag="bwdT_t")
                    nc.sync.dma_start(
                        out=bwdT_t[:], in_=bwdT2d[:, t * 512:(t + 1) * 512])

                    fov = pp.tile([128, 512], F32, tag="fov")
                    nc.tensor.matmul(out=fov[:], lhsT=wfoh_f, rhs=fwdT_t[:],
                                     start=True, stop=False)
                    nc.tensor.matmul(out=fov[:], lhsT=wfoh_b, rhs=bwdT_t[:],
                                     start=False, stop=True)
                    h1 = wp.tile([128, 512], F16, tag="h1")
                    nc.scalar.activation(
                        out=h1[:], in_=fov[:],
                        func=mybir.ActivationFunctionType.Tanh,
                        bias=bias_h,
                    )

                    mov = pp.tile([128, 512], F32, tag="mov")
                    nc.tensor.matmul(out=mov[:], lhsT=wfom_f, rhs=fwdT_t[:],
                                     start=True, stop=False)
                    nc.tensor.matmul(out=mov[:], lhsT=wfom_b, rhs=bwdT_t[:],
                                     start=False, stop=True)
                    h1m = wp.tile([128, 512], F16, tag="h1m")
                    nc.scalar.activation(
                        out=h1m[:], in_=mov[:],
                        func=mybir.ActivationFunctionType.Tanh,
                        bias=bias_m,
                    )

                    h2p = pp.tile([128, 512], F32, tag="h2p")
                    nc.tensor.matmul(out=h2p[:], lhsT=rh2_a, rhs=h1[:],
                                     start=True, stop=False)
                    nc.tensor.matmul(out=h2p[:], lhsT=rh2_b, rhs=h1m[:],
                                     start=False, stop=True)
                    h2s = wp.tile([128, 512], F16, tag="h2s")
                    nc.scalar.activation(
                        out=h2s[:], in_=h2p[:],
                        func=mybir.ActivationFunctionType.Tanh,
                        bias=bias_2,
                    )

                    scp = pp2.tile([64, 512], F32, tag="scp")
                    nc.tensor.matmul(out=scp[:], lhsT=rout_t, rhs=h2s[:],
                                     start=True, stop=True)
                    ssb = wp.tile([64, 512], F32, tag="ssb")
                    nc.scalar.activation(
                        out=ssb[:], in_=scp[:],
                        func=mybir.ActivationFunctionType.Identity,
                        bias=bias_r,
                    )

                    # scores back to [edge, label] layout
                    stp = pp.tile([128, 4 * R], F32, tag="stp")
                    for k in range(4):
                        nc.tensor.transpose(
                            out=stp[:, k * R:(k + 1) * R],
                            in_=ssb[:, k * 128:(k + 1) * 128],
                            identity=ident[0:64, 0:64],
                        )
                    st3 = stp[:].rearrange("p (j r) -> p j r", r=R)

                    # hinge on VectorE
                    relx = rels_sb[:, 4 * t:4 * t + 4].to_broadcast([128, 4, R])
                    mask = wp.tile([128, 4 * R], F32, tag="mask")
                    nc.vector.tensor_tensor(
                        out=mask[:].rearrange("p (j r) -> p j r", r=R),
                        in0=iota_t[:].rearrange("p (j r) -> p j r", r=R),
                        in1=relx,
                        op=mybir.AluOpType.is_equal,
                    )
                    m3 = mask[:].rearrange("p (j r) -> p j r", r=R)
                    gmul = wp.tile([128, 4 * R], F32, tag="gmul")
                    nc.vector.tensor_tensor(
                        out=gmul[:].rearrange("p (j r) -> p j r", r=R),
                        in0=st3, in1=m3, op=mybir.AluOpType.mult,
                    )
                    gold = wp.tile([128, 4], F32, tag="gold")
                    nc.vector.reduce_sum(
                        out=gold[:], in_=gmul[:].rearrange("p (j r) -> p j r", r=R),
                        axis=mybir.AxisListType.X,
                    )
                    wm = wp.tile([128, 4 * R], F32, tag="wm")
                    nc.vector.scalar_tensor_tensor(
                        out=wm[:].rearrange("p (j r) -> p j r", r=R),
                        in0=m3, scalar=-1e30, in1=st3,
                        op0=mybir.AluOpType.mult, op1=mybir.AluOpType.add,
                    )
                    wrong = wp.tile([128, 4], F32, tag="wrong")
                    nc.vector.reduce_max(
                        out=wrong[:], in_=wm[:].rearrange("p (j r) -> p j r", r=R),
                        axis=mybir.AxisListType.X,
                    )
                    dtile = wp.tile([128, 4], F32, tag="dtile")
                    nc.vector.tensor_tensor(
                        out=dtile[:], in0=wrong[:], in1=gold[:],
                        op=mybir.AluOpType.subtract,
                    )
                    nc.vector.scalar_tensor_tensor(
                        out=lerr_acc[:, 4 * t:4 * t + 4],
                        in0=dtile[:], scalar=-1.0, in1=dtile[:],
                        op0=mybir.AluOpType.is_gt, op1=mybir.AluOpType.mult,
                    )
                    t_global += 1
                slot += gn

            # ---- write out lerrs (transpose to edge-major, bf16) ----
            for a in range(0, NB, 128):
                cols = min(128, NB - a)
                otp = pp.tile([128, 128], F32, tag="otp")
                nc.tensor.transpose(
                    out=otp[0:cols, :],
                    in_=lerr_acc[:, a:a + cols],
                    identity=ident[:],
                )
                osb = wp.tile([128, 128], BF16, tag="osb")
                nc.scalar.copy(out=osb[0:cols, :], in_=otp[0:cols, :])
                nc.sync.dma_start(out=lerr_v[a:a + cols, :], in_=osb[0:cols, :])

    nc.compile()
    return nc


_NC_CACHE = {}


def _get_nc():
    if "nc" not in _NC_CACHE:
        _NC_CACHE["nc"] = build_kernel()
    return _NC_CACHE["nc"]


def prepare_weights(WFOH, WFOM, rcatBias, rhid2Layer, rhid2Bias, routLayer,
                    routBias):
    """Pack all MLP weights into one fp16 [128, WCOLS] blob (wall) and the
    biases into the first 4 columns of the f32 ball blob."""
    wall = np.empty((128, WCOLS), dtype=np.float16)
    wall[:, 0:128] = np.asarray(WFOH, np.float16)[0:128]
    wall[:, 128:256] = np.asarray(WFOH, np.float16)[128:256]
    wall[:, 256:384] = np.asarray(WFOM, np.float16)[0:128]
    wall[:, 384:512] = np.asarray(WFOM, np.float16)[128:256]
    wall[:, 512:640] = np.asarray(rhid2Layer, np.float16)[0:128]
    wall[:, 640:768] = np.asarray(rhid2Layer, np.float16)[128:256]
    wall[:, 768:832] = np.asarray(routLayer, np.float16)

    bcols = np.zeros((128, 4), dtype=np.float32)
    bcat = np.asarray(rcatBias, np.float32).reshape(-1)
    bcols[:, 0] = bcat[0:128]
    bcols[:, 1] = bcat[128:256]
    bcols[:, 2] = np.asarray(rhid2Bias, np.float32).reshape(-1)
    bcols[0:64, 3] = np.asarray(routBias, np.float32).reshape(-1)
    return dict(wall=wall, bcols=bcols)


def prepare_core_inputs(fwd, bwd, gold_heads, gold_rels, weights):
    """Bucket edges by head shard; dedup head rows per core; gather modifier
    rows into bucket order; pack everything into one fp16 blob per core.
    Returns (in_maps, edge_lists) where edge_lists[c] maps the core's slot
    i -> global edge id (for output scatter)."""
    fwd16 = np.asarray(fwd, dtype=np.float16)
    bwd16 = np.asarray(bwd, dtype=np.float16)
    heads = np.asarray(gold_heads, dtype=np.int64)
    rels = np.asarray(gold_rels, dtype=np.int64)
    shard = heads >> 15

    in_maps = []
    edge_lists = []
    for c in range(NCORES):
        edges_c = np.nonzero(shard == c)[0]
        cnt = len(edges_c)
        if cnt > CAP:
            raise OverflowError(f"core {c}: {cnt} edges > CAP {CAP}")

        u, inv = np.unique(heads[edges_c], return_inverse=True)
        if len(u) > UCAP:
            raise OverflowError(f"core {c}: {len(u)} unique rows > UCAP {UCAP}")

        blob = np.zeros(TOT, dtype=np.float16)

        bwdT = blob[OFF_BWDT:OFF_BWDT + 128 * CAP].reshape(128, CAP)
        bwdT[:, :cnt] = bwd16[edges_c + 1].T

        fwdu = blob[OFF_FWDU:OFF_FWDU + UCAP * 128].reshape(UCAP, 128)
        fwdu[:len(u)] = fwd16[u]

        idx_local = np.zeros(CAP, dtype=np.int16)
        idx_local[:cnt] = inv.astype(np.int16)
        idx16 = np.tile(np.ascontiguousarray(idx_local.reshape(IDXC, 16).T),
                        (8, 1))
        blob[OFF_IDX:OFF_IDX + 128 * IDXC] = idx16.view(np.float16).reshape(-1)

        blob[OFF_WALL:OFF_WALL + 128 * WCOLS] = weights["wall"].reshape(-1)

        ball = np.empty((128, BCOLS), dtype=np.float32)
        ball[:, 0:4] = weights["bcols"]
        rels_pad = np.zeros(CAP, dtype=np.float32)
        rels_pad[:cnt] = rels[edges_c]
        ball[:, 4:] = rels_pad.reshape(NB, 128).T
        blob[OFF_BALL:] = ball.view(np.float16).reshape(-1)

        in_maps.append(dict(blob=blob))
        edge_lists.append(edges_c)
    return in_maps, edge_lists


def assemble_output(results, edge_lists):
    lerr_full = np.zeros(E, dtype=np.float32)
    for c in range(NCORES):
        out = np.asarray(results[c]["lerr"]).astype(np.float32)
        lerr_full[edge_lists[c]] = out[:len(edge_lists[c])]
    return lerr_full


def kernel(fwd, bwd, gold_heads, gold_rels, WFOH, WFOM, rhidBias, rcatBias,
           rhid2Layer, rhid2Bias, routLayer, routBias):
    nc = _get_nc()
    weights = prepare_weights(WFOH, WFOM, rcatBias, rhid2Layer, rhid2Bias,
                              routLayer, routBias)
    in_maps, edge_lists = prepare_core_inputs(fwd, bwd, gold_heads, gold_rels,
                                              weights)
    res = run_bass_kernel_spmd(nc, in_maps, list(range(NCORES)))
    return assemble_output(res.results, edge_lists)
